# revision 29
# baseline (speedup 1.0000x reference)
"""GQA causal prefill attention on 8 TRN2 NeuronCores.

Sharding: head-parallel. Core c computes q heads [4c, 4c+4) against kv head c
(n_rep = 4, so GQA groups align with the shard; no cross-core communication).

Host prep (per core): q sliced + transposed to qT (4, 128, 2048) bf16,
k transposed to kT (128, 2048) bf16, v kept (2048, 128) bf16.  This removes
all on-device PE transposes and DVE casts and halves input DMA bytes.

Device algorithm per core (T=2048, 4 q heads, head_dim 128, P=128 tiles):
  - S^T tiles: matmul(lhsT=kT_j, rhs=qT_h tiles i>=j) -> PSUM, f32.
  - The per-head stream of 136 S^T tiles is greedily packed into alternating
    4-bank / 3-bank PSUM units (16 / 12 tiles); ONE scalar-engine activation
    (exp, scale folded in) per unit writes bf16 e^T into a per-head packed
    SBUF buffer (stream-ordered offsets).  ~10 ACTIVATEs per head instead of
    20+ cuts the ACT pipeline-fill overhead; ACT is a hard bottleneck
    (1 col/cycle @ 1.2 GHz, ~58us of exp per core).
  - Causal mask on diagonal tiles: upper-triangular multiply on DVE.
  - PV per (head, t-tile i): e^T blocks stationary, v_aug (v | ones) streams;
    psum (t, 129) accumulates over j; col 128 is the softmax denominator.
    DVE reciprocal + per-partition multiply normalizes into an SBUF staging
    tile; output DMAs are batched 4 t-tiles at a time.
  - PV chains are interleaved between QK units under a PE-vs-ACT cycle
    budget so both engines stay busy; head 0 walks j = 15..8 then 0..7 so
    compute starts after only the first DMA chunks land.
"""

import sys
import functools

import numpy as np

if "/opt/trn_rl_repo" not in sys.path:
    sys.path.insert(0, "/opt/trn_rl_repo")

T = 2048
H_TOTAL = 32
N_CORES = 8
H = H_TOTAL // N_CORES  # 4 q heads per core
D = 128
P = 128
NT = T // P  # 16 token tiles
SCALE = 0.08838834764831845

UNIT_CAPS = (16, 12)  # alternating PSUM unit sizes in tiles (4 banks / 3 banks)


def _j_order(h):
    if h == 0:
        return list(range(NT - 1, 7, -1)) + list(range(0, 8))
    return list(range(NT))


def _pack_units(j_order):
    """Greedily pack the S^T tile stream (j-major, i ascending within j) into
    units of alternating capacity.  Returns (units, eoff, ready_unit):
      units: list of runs [(j, i0, n, p0), ...] with p0 = tile pos in unit
      eoff:  eoff[j] = tile position of block (i=j, j) within the per-head
             packed e^T buffer (stream-ordered)
      ready_unit[i]: index of the unit whose exp completes chain i
    """
    stream = []  # (j, i)
    eoff = [0] * NT
    for j in j_order:
        eoff[j] = len(stream)
        for i in range(j, NT):
            stream.append((j, i))
    # block position of (i, j): eoff[j] + (i - j)
    units = []
    pos = 0
    u = 0
    while pos < len(stream):
        cap = UNIT_CAPS[u % 2]
        n = min(cap, len(stream) - pos)
        runs = []
        p = pos
        while p < pos + n:
            j, i0 = stream[p]
            m = 1
            while p + m < pos + n and stream[p + m] == (j, i0 + m):
                m += 1
            runs.append((j, i0, m, p - pos))
            p += m
        units.append(runs)
        pos += n
        u += 1
    # unit index covering each stream position
    unit_of_pos = []
    for ui, runs in enumerate(units):
        unit_of_pos.extend([ui] * sum(r[2] for r in runs))
    ready_unit = []
    for i in range(NT):
        last = max(eoff[j] + (i - j) for j in range(i + 1))
        ready_unit.append(unit_of_pos[last])
    return units, eoff, ready_unit


def _build_body(tc, nc, q_d, k_d, v_d, o_d, ctx):
    from collections import deque

    import concourse.mybir as mybir
    from concourse.masks import make_identity, make_lower_triangular

    f32 = mybir.dt.float32
    bf16 = mybir.dt.bfloat16

    const = ctx.enter_context(tc.tile_pool(name="const", bufs=1))
    qtp = ctx.enter_context(tc.tile_pool(name="qT", bufs=H))
    ep = ctx.enter_context(tc.tile_pool(name="eT", bufs=2))
    outp = ctx.enter_context(tc.tile_pool(name="outsb", bufs=2))
    recp = ctx.enter_context(tc.tile_pool(name="rec", bufs=4))

    # PSUM: 4-bank + 3-bank S^T units (ping-pong) + 1 bank holding two
    # interleaved PV accumulator slots = exactly 8 banks.
    stx_pool = ctx.enter_context(tc.tile_pool(name="stx", bufs=1, space="PSUM"))
    stb_pool = ctx.enter_context(tc.tile_pool(name="stb", bufs=1, space="PSUM"))
    pv_pool = ctx.enter_context(tc.tile_pool(name="pv", bufs=1, space="PSUM"))

    # Causal masking happens on the PE: one extra matmul accumulates a
    # strictly-lower-triangular -1e4 into each diagonal S^T tile in PSUM, so
    # exp() zeroes the masked half and no cross-engine mask dependency exists.
    identity = const.tile([P, P], bf16, tag="ident")
    make_identity(nc, identity)
    mlow = const.tile([P, P], bf16, tag="mlow")
    make_lower_triangular(nc, mlow, val=-500.0, diag=False)

    # SBUF-resident operands (bf16, pre-transposed on host).
    kT = const.tile([P, NT, P], bf16, tag="kT")        # [d, j, s]
    v_aug = const.tile([P, NT, D + 1], bf16, tag="vaug")  # [s, j, d|1]
    qT = [qtp.tile([P, NT, P], bf16, tag="qT", name=f"qT{h}") for h in range(H)]

    nc.vector.memset(v_aug[:, :, D:D + 1], 1.0)

    # Input DMAs spread over four engine DGE queues so transfers run in
    # parallel (a single ring moves ~70 GB/s; serial staging took ~15us).
    # kT/qT0 tail halves first -- head 0 walks j descending, so compute
    # starts as soon as the first half-tensors land.
    k_view = k_d.rearrange("d (j p) -> d j p", p=P)
    q_view = q_d.rearrange("h d (i p) -> h d i p", p=P)
    v_view = v_d.rearrange("(j p) d -> p j d", p=P)
    nc.sync.dma_start(kT[:, 12:16, :], k_view[:, 12:16, :])
    nc.sync.dma_start(kT[:, 8:12, :], k_view[:, 8:12, :])
    nc.sync.dma_start(kT[:, 0:8, :], k_view[:, 0:8, :])
    nc.scalar.dma_start(qT[0][:, 12:16, :], q_view[0, :, 12:16, :])
    nc.scalar.dma_start(qT[0][:, 8:12, :], q_view[0, :, 8:12, :])
    nc.scalar.dma_start(qT[0][:, 0:8, :], q_view[0, :, 0:8, :])
    nc.sync.dma_start(v_aug[:, :, 0:D], v_view)
    nc.gpsimd.dma_start(qT[1], q_view[1])
    nc.gpsimd.dma_start(qT[2], q_view[2])
    nc.gpsimd.dma_start(qT[3], q_view[3])

    # Prewarm the ACT exp table so the ~1.3us load happens during DMA wait
    # (after the scalar queue's DMA issues so staging isn't delayed).
    warm_sb = recp.tile([P, 1], f32, tag="rec", name="warm")
    nc.scalar.activation(
        out=warm_sb, in_=identity[:, 0:1],
        func=mybir.ActivationFunctionType.Exp,
    )

    o_view = o_d.rearrange("(i p) h d -> p i h d", p=P)

    eT = [None, None]  # per-slot eT tiles (bufs=2 rotation)
    eoffs = [None] * H

    pv_bank = pv_pool.tile([P, 3, D + 1], f32, tag="pv")
    chain_ct = [0]  # global chain counter for pv slot rotation
    # main-phase slots; the drain phase adds slots carved from the S^T units
    pv_slots = [pv_bank[:, s, :] for s in range(3)]

    out_sb = [None, None]

    ready = deque()  # (h, i) chains whose e^T blocks are all available

    # PE-vs-ACT cycle budget (PE cycles; ACT runs at half the PE clock).
    pe_cyc = [0.0]
    act_cyc = [0.0]
    PV_BLOCK_CYC = 129 + 70
    QK_MM_OV = 25
    EXP_OV = 352
    SLACK = 1000.0

    def emit_chain(h, i):
        eo = eoffs[h]
        et = eT[h % 2]
        slot = chain_ct[0] % len(pv_slots)
        chain_ct[0] += 1
        pv = pv_slots[slot]
        for j in range(i + 1):
            c0 = (eo[j] + (i - j)) * P
            nc.tensor.matmul(
                pv,
                lhsT=et[:, c0:c0 + P],
                rhs=v_aug[:, j, :],
                start=(j == 0),
                stop=(j == i),
            )
        pe_cyc[0] += (i + 1) * PV_BLOCK_CYC
        rec = recp.tile([P, 1], f32, tag="rec")
        nc.vector.reciprocal(rec, pv[:, D:D + 1])
        nc.vector.tensor_scalar_mul(out_sb[h % 2][:, i, :], pv[:, 0:D], rec)
        # batch output DMAs 4 tiles at a time; the last head goes 2 at a time
        # so the final transfer (the serial tail) is small
        g = 2 if h == H - 1 else 4
        if i % g == g - 1:
            eng = nc.sync if (h * 8 + i // g) % 2 == 0 else nc.gpsimd
            eng.dma_start(
                o_view[:, i - g + 1:i + 1, h, :],
                out_sb[h % 2][:, i - g + 1:i + 1, :],
            )

    def pop_chains(budget_fn, force=False):
        while ready and (force or budget_fn()):
            h2, i2 = ready.popleft()
            emit_chain(h2, i2)

    for h in range(H):
        units, eoff_tiles, ready_unit = _pack_units(_j_order(h))
        eoffs[h] = eoff_tiles
        # drain chains still pointing at the eT slot this head will overwrite
        while ready and ready[0][0] <= h - 2:
            h2, i2 = ready.popleft()
            emit_chain(h2, i2)
        eT[h % 2] = ep.tile([P, 136 * P], bf16, tag="eT", name="eT")
        out_sb[h % 2] = outp.tile([P, NT, D], bf16, tag="outsb", name="outsb")
        for ui, runs in enumerate(units):
            ntiles = sum(r[2] for r in runs)
            pool = stx_pool if ui % 2 == 0 else stb_pool
            cap = UNIT_CAPS[ui % 2]
            stu = pool.tile([P, cap * P], f32, tag="st")
            for (j, i0, n, p0) in runs:
                diag_p0 = p0 if i0 == j else None
                # chunk <=4 tiles, never crossing a 512-col PSUM bank line
                while n > 0:
                    m = min(4 - (p0 % 4), n)
                    nc.tensor.matmul(
                        stu[:, p0 * P:(p0 + m) * P],
                        lhsT=kT[:, j, :],
                        rhs=qT[h][:, i0:i0 + m, :],
                        start=True,
                        stop=True,
                    )
                    pe_cyc[0] += m * P + QK_MM_OV
                    if diag_p0 is not None:
                        # causal mask: accumulate -500 above the causal
                        # boundary straight into PSUM.  Must follow the
                        # diag-writing chunk IMMEDIATELY: a later start=True
                        # in the same bank re-marks the whole 2KB zero
                        # region as pending-zero, which would turn this
                        # accumulate into an overwrite.
                        nc.tensor.matmul(
                            stu[:, diag_p0 * P:(diag_p0 + 1) * P],
                            lhsT=identity,
                            rhs=mlow,
                            start=False,
                            stop=True,
                            skip_group_check=True,
                        )
                        pe_cyc[0] += P + 70
                        diag_p0 = None
                    p0 += m
                    i0 += m
                    n -= m
            upos = sum(
                sum(r[2] for r in units[k]) for k in range(ui)
            )
            nc.scalar.activation(
                out=eT[h % 2][:, upos * P:(upos + ntiles) * P],
                in_=stu[:, 0:ntiles * P],
                func=mybir.ActivationFunctionType.Exp,
                scale=SCALE,
            )
            act_cyc[0] += ntiles * P + EXP_OV
            for i in range(NT):
                if ready_unit[i] == ui:
                    ready.append((h, i))
            # PV filler AFTER the unit's MMs + exp: ACT gets fed as early as
            # possible, then chains keep the PE busy while ACT chews the exp.
            pop_chains(lambda: pe_cyc[0] + 400 < 2 * act_cyc[0] + SLACK)
    # Final drain: no more QK, so repurpose the S^T PSUM units as extra PV
    # accumulator slots -- DVE normalizes pipeline behind the PE instead of
    # stalling it through the 3-slot rotation.
    if ready:
        drain_x = stx_pool.tile([P, UNIT_CAPS[0] * P], f32, tag="st")
        drain_b = stb_pool.tile([P, UNIT_CAPS[1] * P], f32, tag="st")
        for tile_, banks in ((drain_x, 4), (drain_b, 3)):
            for b in range(banks):
                for s in range(3):
                    c0 = b * 512 + s * (D + 1)
                    pv_slots.append(tile_[:, c0:c0 + D + 1])
    pop_chains(None, force=True)


@functools.lru_cache(maxsize=1)
def _build():
    import concourse.tile as tile
    import concourse.mybir as mybir
    from concourse import bacc
    from contextlib import ExitStack

    f32 = mybir.dt.float32
    bf16 = mybir.dt.bfloat16
    nc = bacc.Bacc(
        "TRN2",
        target_bir_lowering=False,
        debug=False,
        num_devices=N_CORES,
    )
    q_d = nc.dram_tensor("q", (H, D, T), bf16, kind="ExternalInput").ap()
    k_d = nc.dram_tensor("k", (D, T), bf16, kind="ExternalInput").ap()
    v_d = nc.dram_tensor("v", (T, D), bf16, kind="ExternalInput").ap()
    o_d = nc.dram_tensor("out", (T, H, D), bf16, kind="ExternalOutput").ap()

    with tile.TileContext(nc) as tc:
        with ExitStack() as ctx:
            _build_body(tc, nc, q_d, k_d, v_d, o_d, ctx)
    nc.compile()
    return nc


def _in_maps(q, k, v):
    import ml_dtypes

    bf16 = ml_dtypes.bfloat16
    q = np.asarray(q, dtype=np.float32)
    k = np.asarray(k, dtype=np.float32)
    v = np.asarray(v, dtype=np.float32)
    maps = []
    for c in range(N_CORES):
        qt = np.ascontiguousarray(
            q[:, H * c:H * c + H, :].transpose(1, 2, 0)
        ).astype(bf16)  # (H, D, T)
        kt = np.ascontiguousarray(k[:, c, :].T).astype(bf16)  # (D, T)
        vc = np.ascontiguousarray(v[:, c, :]).astype(bf16)  # (T, D)
        maps.append({"q": qt, "k": kt, "v": vc})
    return maps


def kernel(q, k, v, _trace=False):
    from concourse.bass_utils import run_bass_kernel_spmd

    nc = _build()
    res = run_bass_kernel_spmd(
        nc, _in_maps(q, k, v), core_ids=list(range(N_CORES)), trace=_trace
    )
    out = np.empty((T, H_TOTAL, D), dtype=np.float32)
    for c in range(N_CORES):
        out[:, H * c:H * c + H, :] = np.asarray(
            res.results[c]["out"], dtype=np.float32
        ).reshape(T, H, D)
    if _trace:
        return out, res
    return out
